# revision 38
# baseline (speedup 1.0000x reference)
"""MBart expert-layer (MoE routing) kernel for 8 Trainium2 NeuronCores.

Strategy: data-parallel over batch. Each batch row routes to exactly one
expert (lang code), so the expert gather happens on host (langs are host
data).  Core b computes a dense SwiGLU MLP for row b:
    out = (gelu(x @ W1) * (x @ W3)) @ W2
All device work happens in transposed orientation (activations stored
[d_model, seq]) so both matmul stages use the natural [K, M] weight layouts
as the stationary operand and no on-device transposes are needed.
Matmul inputs are bf16 (fp32 accumulate in PSUM); gelu/mul in fp32.

The TPB ISA allows one sync wait per instruction and this walrus build
refuses multi-wait instructions, so the module is built as bacc.Bacc and
nc.compile() runs bacc's generate_event_semaphores pass, which splits
excess waits into event-semaphore chains.  build_nc() asserts the
resulting ≤1-wait invariant.
"""

import numpy as np
import ml_dtypes
from contextlib import ExitStack

import concourse.bass as bass
import concourse.bacc as bacc
import concourse.mybir as mybir
from concourse.tile import TileContext
from concourse.bass_utils import run_bass_kernel_spmd

E, B, S, D, F = 8, 8, 2048, 1024, 4096
LANG_BASE = 4
P = 128
MT = 512          # matmul moving free dim (seq chunk)
NG = 2            # seq super-chunks; weights streamed NG times
BF16 = mybir.dt.bfloat16
F32 = mybir.dt.float32
bf16 = ml_dtypes.bfloat16


def build_nc(S_=S, D_=D, F_=F, MT_=MT, NG_=NG,
             act=mybir.ActivationFunctionType.Gelu, check_waits=True):
    DT, FT = D_ // P, F_ // P
    sg = S_ // NG_
    nm = sg // MT_
    W2SUB = 8 if FT % 8 == 0 else 1   # w2 block split into sub-DMAs per d_i
    FS = FT // W2SUB                  # f-tiles per w2 sub-block
    nc = bacc.Bacc()
    xt = nc.declare_dram_parameter("xt", [DT, P, S_], BF16, isOutput=False)
    w1 = nc.declare_dram_parameter("w1", [FT, P, DT, P], BF16, isOutput=False)
    w3 = nc.declare_dram_parameter("w3", [FT, P, DT, P], BF16, isOutput=False)
    w2 = nc.declare_dram_parameter("w2", [DT, P, FT, P], BF16, isOutput=False)
    ot = nc.declare_dram_parameter("ot", [DT, P, S_], F32, isOutput=True)

    with TileContext(nc) as tc, ExitStack() as ctx:
        xpool = ctx.enter_context(tc.tile_pool(name="x", bufs=1))
        wpool = ctx.enter_context(tc.tile_pool(name="w", bufs=4))
        w2pool = ctx.enter_context(tc.tile_pool(name="w2", bufs=2))
        hpool = ctx.enter_context(tc.tile_pool(name="h", bufs=1))
        gpool = ctx.enter_context(tc.tile_pool(name="g", bufs=3))
        opool = ctx.enter_context(tc.tile_pool(name="o", bufs=3))
        ppool = ctx.enter_context(tc.tile_pool(name="ps", bufs=2, space="PSUM"))
        p2pool = ctx.enter_context(tc.tile_pool(name="ps2", bufs=2, space="PSUM"))

        x_sb = []
        for d_i in range(DT):
            t = xpool.tile([P, S_], BF16, name=f"x{d_i}", tag=f"x{d_i}")
            x_sb.append(t)
        # Load the first super-chunk's columns of every x tile before the
        # rest so the first matmuls stop waiting on the full 4MB transfer.
        # Each dma_start costs ~600ns of issuing-queue time (DIRECT2D), so
        # eight triggers on one queue serialize over ~5us -- spread the
        # first super-chunk's triggers across the queues that are idle at
        # kernel start (vector/scalar) to get all eight transfers in
        # flight by ~7.5us instead of ~10.5us.
        first_q = [nc.sync, nc.sync, nc.sync, nc.sync, nc.scalar,
                   nc.scalar, nc.scalar, nc.scalar]
        for d_i in range(DT):
            first_q[d_i % len(first_q)].dma_start(
                out=x_sb[d_i][:, 0:sg], in_=xt[d_i][:, 0:sg])


        for g in range(NG_):
            s0 = g * sg
            # ---- phase A: hT[f, m] = gelu(W1.T x) * (W3.T x) ----
            h_tiles = []
            for f_i in range(FT):
                w1_t = wpool.tile([P, DT, P], BF16, name="w1t", tag="w1t")
                w3_t = wpool.tile([P, DT, P], BF16, name="w3t", tag="w3t")
                # Halved weight transfers lower the instantaneous SBUF
                # write burst while a tile lands (suspected source of
                # periodic PE operand-fetch stalls).
                hd = DT // 2
                nc.gpsimd.dma_start(out=w1_t[:, :hd, :], in_=w1[f_i][:, :hd, :])
                nc.gpsimd.dma_start(out=w1_t[:, hd:, :], in_=w1[f_i][:, hd:, :])
                nc.gpsimd.dma_start(out=w3_t[:, :hd, :], in_=w3[f_i][:, :hd, :])
                nc.gpsimd.dma_start(out=w3_t[:, hd:, :], in_=w3[f_i][:, hd:, :])
                h_sb = hpool.tile([P, sg], BF16, name=f"h{f_i}", tag=f"h{f_i}")
                for m in range(nm):
                    ms = s0 + m * MT_
                    a_ps = ppool.tile([P, MT_], F32, name="a_ps", tag="a")
                    b_ps = ppool.tile([P, MT_], F32, name="b_ps", tag="b")
                    # b (w3 path) first so the gelu is the latest producer
                    # feeding the h-mul: the wait legalizer can then anchor
                    # the mul's PE wait on the gelu at zero cost.
                    # For the first f-iteration, accumulate in x ARRIVAL
                    # order (d0-d3 stream via sync, d4-d7 via scalar, so
                    # chunks land interleaved): consuming in-order leaves
                    # micro-gaps that keep the early-stream busy fraction
                    # under the HAM un-throttle threshold on unlucky runs.
                    if g == 0 and f_i == 0:
                        dord = [0, 4, 1, 5, 2, 6, 3, 7][:DT] if DT == 8                             else list(range(DT))
                    else:
                        dord = list(range(DT))
                    for i, d_i in enumerate(dord):
                        nc.tensor.matmul(
                            b_ps[:], w3_t[:, d_i, :], x_sb[d_i][:, ms:ms + MT_],
                            start=(i == 0), stop=(i == DT - 1))
                    for i, d_i in enumerate(dord):
                        nc.tensor.matmul(
                            a_ps[:], w1_t[:, d_i, :], x_sb[d_i][:, ms:ms + MT_],
                            start=(i == 0), stop=(i == DT - 1))
                    g_sb = gpool.tile([P, MT_], F32, name="g_sb", tag="g")
                    nc.scalar.activation(g_sb[:], a_ps[:], act)
                    # Later seq super-chunks of x are triggered one-per-gelu
                    # here: the scalar queue reaches these only after the
                    # first gelu (~17.5us), so the g1 transfers stop
                    # fair-sharing the DMA rings with the critical first
                    # 2.5MB (they were slowing it ~45%; the first matmul is
                    # purely data-gated on that set).
                    gi = (2 * f_i + m) * NG_ // 8 if NG_ > 1 else 0
                    if g == 0 and 2 * f_i + m < 8 * (NG_ - 1):
                        slot = 2 * f_i + m
                        g2, d2 = 1 + slot // DT, slot % DT
                        nc.scalar.dma_start(
                            out=x_sb[d2][:, g2 * sg:(g2 + 1) * sg],
                            in_=xt[d2][:, g2 * sg:(g2 + 1) * sg])
                    nc.vector.tensor_mul(
                        h_sb[:, m * MT_:(m + 1) * MT_], g_sb[:], b_ps[:])
                h_tiles.append(h_sb)
            # ---- phase B: outT[d, m] = W2.T hT ----
            for d_i in range(DT):
                w2_ts = []
                for k in range(W2SUB):
                    w2_t = w2pool.tile([P, FS, P], BF16, name=f"w2t{k}",
                                       tag=f"w2t{k}")
                    nc.gpsimd.dma_start(
                        out=w2_t[:], in_=w2[d_i][:, k * FS:(k + 1) * FS, :])
                    w2_ts.append(w2_t)
                for m in range(nm):
                    o_ps = p2pool.tile([P, MT_], F32, name="o_ps", tag="o")
                    for f_i in range(FT):
                        nc.tensor.matmul(
                            o_ps[:], w2_ts[f_i // FS][:, f_i % FS, :],
                            h_tiles[f_i][:, m * MT_:(m + 1) * MT_],
                            start=(f_i == 0), stop=(f_i == FT - 1))
                    o_sb = opool.tile([P, MT_], F32, name="o_sb", tag="osb")
                    # Two-way split pipelines the copy with the first DMA
                    # and roughly halves the copy->DMA tail after the very
                    # last matmul.
                    oc = MT_ // 2
                    last = (g == NG_ - 1 and d_i == DT - 1 and m == nm - 1)
                    # On the final chunk, fire the two half-DMAs from
                    # different queues so their ~600ns trigger executions
                    # overlap instead of serializing on the tail.
                    oq = [nc.sync, nc.scalar] if last else [nc.sync, nc.sync]
                    for k in range(2):
                        if last and k == 1:
                            # Copy the second half on the scalar engine so
                            # both copies (and both DMA triggers) overlap on
                            # the critical tail after the final matmul.
                            nc.scalar.copy(
                                o_sb[:, k * oc:(k + 1) * oc],
                                o_ps[:, k * oc:(k + 1) * oc])
                        else:
                            nc.vector.tensor_copy(
                                o_sb[:, k * oc:(k + 1) * oc],
                                o_ps[:, k * oc:(k + 1) * oc])
                        oq[k].dma_start(
                            out=ot[d_i][:, s0 + m * MT_ + k * oc:
                                        s0 + m * MT_ + (k + 1) * oc],
                            in_=o_sb[:, k * oc:(k + 1) * oc])

    nc.compile()
    if check_waits:
        skip = ("InstDrain", "InstEventSemaphore")
        bad = []
        for f in nc.m.functions:
            for bb in f.blocks:
                for inst in bb.instructions:
                    if type(inst).__name__ in skip or inst.sync_info is None:
                        continue
                    nw = len(inst.sync_info.on_wait or [])
                    if nw > 1:
                        bad.append((inst.name, type(inst).__name__, nw))
        if bad:
            raise RuntimeError(f"insts with >1 wait: {bad[:8]}")
    return nc


_NC_CACHE = {}


def _get_nc():
    if "nc" not in _NC_CACHE:
        _NC_CACHE["nc"] = build_nc()
    return _NC_CACHE["nc"]


def make_in_maps(hidden_states, w1, w2, w3, langs):
    hs = np.asarray(hidden_states, np.float32)
    w1 = np.asarray(w1, np.float32)
    w2 = np.asarray(w2, np.float32)
    w3 = np.asarray(w3, np.float32)
    langs = np.asarray(langs)
    DT, FT = D // P, F // P
    in_maps = []
    for b in range(B):
        e = int(langs[b, 0] - LANG_BASE) % E
        xtb = np.ascontiguousarray(hs[b].T.astype(bf16)).reshape(DT, P, S)
        w1b = np.ascontiguousarray(
            w1[e].reshape(DT, P, FT, P).transpose(2, 1, 0, 3).astype(bf16))
        w3b = np.ascontiguousarray(
            w3[e].reshape(DT, P, FT, P).transpose(2, 1, 0, 3).astype(bf16))
        w2b = np.ascontiguousarray(
            w2[e].reshape(FT, P, DT, P).transpose(2, 1, 0, 3).astype(bf16))
        in_maps.append({"xt": xtb, "w1": w1b, "w3": w3b, "w2": w2b})
    return in_maps


def assemble_output(results):
    out = np.empty((B, S, D), np.float32)
    for b in range(B):
        out[b] = results[b]["ot"].reshape(D, S).T
    return out


def kernel(hidden_states, w1, w2, w3, langs, **kw):
    nc = _get_nc()
    in_maps = make_in_maps(hidden_states, w1, w2, w3, langs)
    res = run_bass_kernel_spmd(nc, in_maps, list(range(8)))
    return assemble_output(res.results)


if __name__ == "__main__":
    rng = np.random.default_rng(0)
    hs = rng.standard_normal((B, S, D)).astype(np.float32)
    w1_ = (rng.standard_normal((E, D, F)) / np.sqrt(D)).astype(np.float32)
    w3_ = (rng.standard_normal((E, D, F)) / np.sqrt(D)).astype(np.float32)
    w2_ = (rng.standard_normal((E, F, D)) / np.sqrt(F)).astype(np.float32)
    langs = rng.integers(4, 12, (B, 1)).astype(np.int64)
    out = kernel(hs, w1_, w2_, w3_, langs)
    print(out.shape, out.dtype)


# revision 40
# speedup vs baseline: 1.0038x; 1.0038x over previous
"""MBart expert-layer (MoE routing) kernel for 8 Trainium2 NeuronCores.

Strategy: data-parallel over batch. Each batch row routes to exactly one
expert (lang code), so the expert gather happens on host (langs are host
data).  Core b computes a dense SwiGLU MLP for row b:
    out = (gelu(x @ W1) * (x @ W3)) @ W2
All device work happens in transposed orientation (activations stored
[d_model, seq]) so both matmul stages use the natural [K, M] weight layouts
as the stationary operand and no on-device transposes are needed.
Matmul inputs are bf16 (fp32 accumulate in PSUM); gelu/mul in fp32.

The TPB ISA allows one sync wait per instruction and this walrus build
refuses multi-wait instructions, so the module is built as bacc.Bacc and
nc.compile() runs bacc's generate_event_semaphores pass, which splits
excess waits into event-semaphore chains.  build_nc() asserts the
resulting ≤1-wait invariant.
"""

import numpy as np
import ml_dtypes
from contextlib import ExitStack

import concourse.bass as bass
import concourse.bacc as bacc
import concourse.mybir as mybir
from concourse.tile import TileContext
from concourse.bass_utils import run_bass_kernel_spmd

E, B, S, D, F = 8, 8, 2048, 1024, 4096
LANG_BASE = 4
P = 128
MT = 512          # matmul moving free dim (seq chunk)
NG = 2            # seq super-chunks; weights streamed NG times
BF16 = mybir.dt.bfloat16
F32 = mybir.dt.float32
bf16 = ml_dtypes.bfloat16


def build_nc(S_=S, D_=D, F_=F, MT_=MT, NG_=NG,
             act=mybir.ActivationFunctionType.Gelu, check_waits=True):
    DT, FT = D_ // P, F_ // P
    sg = S_ // NG_
    nm = sg // MT_
    W2SUB = 8 if FT % 8 == 0 else 1   # w2 block split into sub-DMAs per d_i
    FS = FT // W2SUB                  # f-tiles per w2 sub-block
    nc = bacc.Bacc()
    xt = nc.declare_dram_parameter("xt", [DT, P, S_], BF16, isOutput=False)
    w1 = nc.declare_dram_parameter("w1", [FT, P, DT, P], BF16, isOutput=False)
    w3 = nc.declare_dram_parameter("w3", [FT, P, DT, P], BF16, isOutput=False)
    w2 = nc.declare_dram_parameter("w2", [DT, P, FT, P], BF16, isOutput=False)
    ot = nc.declare_dram_parameter("ot", [DT, P, S_], F32, isOutput=True)

    with TileContext(nc) as tc, ExitStack() as ctx:
        xpool = ctx.enter_context(tc.tile_pool(name="x", bufs=1))
        wpool = ctx.enter_context(tc.tile_pool(name="w", bufs=2))
        w2pool = ctx.enter_context(tc.tile_pool(name="w2", bufs=2))
        hpool = ctx.enter_context(tc.tile_pool(name="h", bufs=1))
        gpool = ctx.enter_context(tc.tile_pool(name="g", bufs=3))
        opool = ctx.enter_context(tc.tile_pool(name="o", bufs=3))
        ppool = ctx.enter_context(tc.tile_pool(name="ps", bufs=2, space="PSUM"))
        p2pool = ctx.enter_context(tc.tile_pool(name="ps2", bufs=2, space="PSUM"))

        x_sb = []
        for d_i in range(DT):
            t = xpool.tile([P, S_], BF16, name=f"x{d_i}", tag=f"x{d_i}")
            x_sb.append(t)
        # Load the first super-chunk's columns of every x tile before the
        # rest so the first matmuls stop waiting on the full 4MB transfer.
        # Each dma_start costs ~600ns of issuing-queue time (DIRECT2D), so
        # eight triggers on one queue serialize over ~5us -- spread the
        # first super-chunk's triggers across the queues that are idle at
        # kernel start (vector/scalar) to get all eight transfers in
        # flight by ~7.5us instead of ~10.5us.
        first_q = [nc.sync, nc.sync, nc.sync, nc.sync, nc.scalar,
                   nc.scalar, nc.scalar, nc.scalar]
        for d_i in range(DT):
            first_q[d_i % len(first_q)].dma_start(
                out=x_sb[d_i][:, 0:sg], in_=xt[d_i][:, 0:sg])


        for g in range(NG_):
            s0 = g * sg
            # ---- phase A: hT[f, m] = gelu(W1.T x) * (W3.T x) ----
            h_tiles = []
            for f_i in range(FT):
                w1_t = wpool.tile([P, DT, P], BF16, name="w1t", tag="w1t")
                w3_t = wpool.tile([P, DT, P], BF16, name="w3t", tag="w3t")
                # Halved weight transfers lower the instantaneous SBUF
                # write burst while a tile lands (suspected source of
                # periodic PE operand-fetch stalls).
                hd = DT // 2
                nc.gpsimd.dma_start(out=w1_t[:, :hd, :], in_=w1[f_i][:, :hd, :])
                nc.gpsimd.dma_start(out=w1_t[:, hd:, :], in_=w1[f_i][:, hd:, :])
                nc.gpsimd.dma_start(out=w3_t[:, :hd, :], in_=w3[f_i][:, :hd, :])
                nc.gpsimd.dma_start(out=w3_t[:, hd:, :], in_=w3[f_i][:, hd:, :])
                h_sb = hpool.tile([P, sg], BF16, name=f"h{f_i}", tag=f"h{f_i}")
                for m in range(nm):
                    ms = s0 + m * MT_
                    a_ps = ppool.tile([P, MT_], F32, name="a_ps", tag="a")
                    b_ps = ppool.tile([P, MT_], F32, name="b_ps", tag="b")
                    # b (w3 path) first so the gelu is the latest producer
                    # feeding the h-mul: the wait legalizer can then anchor
                    # the mul's PE wait on the gelu at zero cost.
                    # For the first f-iteration, accumulate in x ARRIVAL
                    # order (d0-d3 stream via sync, d4-d7 via scalar, so
                    # chunks land interleaved): consuming in-order leaves
                    # micro-gaps that keep the early-stream busy fraction
                    # under the HAM un-throttle threshold on unlucky runs.
                    if g == 0 and f_i == 0:
                        dord = [0, 4, 1, 5, 2, 6, 3, 7][:DT] if DT == 8                             else list(range(DT))
                    else:
                        dord = list(range(DT))
                    for i, d_i in enumerate(dord):
                        nc.tensor.matmul(
                            b_ps[:], w3_t[:, d_i, :], x_sb[d_i][:, ms:ms + MT_],
                            start=(i == 0), stop=(i == DT - 1))
                    for i, d_i in enumerate(dord):
                        nc.tensor.matmul(
                            a_ps[:], w1_t[:, d_i, :], x_sb[d_i][:, ms:ms + MT_],
                            start=(i == 0), stop=(i == DT - 1))
                    g_sb = gpool.tile([P, MT_], F32, name="g_sb", tag="g")
                    nc.scalar.activation(g_sb[:], a_ps[:], act)
                    # Later seq super-chunks of x are triggered one-per-gelu
                    # here: the scalar queue reaches these only after the
                    # first gelu (~17.5us), so the g1 transfers stop
                    # fair-sharing the DMA rings with the critical first
                    # 2.5MB (they were slowing it ~45%; the first matmul is
                    # purely data-gated on that set).
                    if g == 0 and 2 * f_i + m < 8 * (NG_ - 1):
                        slot = 2 * f_i + m
                        g2, d2 = 1 + slot // DT, slot % DT
                        nc.scalar.dma_start(
                            out=x_sb[d2][:, g2 * sg:(g2 + 1) * sg],
                            in_=xt[d2][:, g2 * sg:(g2 + 1) * sg])
                    nc.vector.tensor_mul(
                        h_sb[:, m * MT_:(m + 1) * MT_], g_sb[:], b_ps[:])
                h_tiles.append(h_sb)
            # ---- phase B: outT[d, m] = W2.T hT ----
            for d_i in range(DT):
                w2_ts = []
                for k in range(W2SUB):
                    w2_t = w2pool.tile([P, FS, P], BF16, name=f"w2t{k}",
                                       tag=f"w2t{k}")
                    nc.gpsimd.dma_start(
                        out=w2_t[:], in_=w2[d_i][:, k * FS:(k + 1) * FS, :])
                    w2_ts.append(w2_t)
                for m in range(nm):
                    o_ps = p2pool.tile([P, MT_], F32, name="o_ps", tag="o")
                    for f_i in range(FT):
                        nc.tensor.matmul(
                            o_ps[:], w2_ts[f_i // FS][:, f_i % FS, :],
                            h_tiles[f_i][:, m * MT_:(m + 1) * MT_],
                            start=(f_i == 0), stop=(f_i == FT - 1))
                    o_sb = opool.tile([P, MT_], F32, name="o_sb", tag="osb")
                    # Two-way split pipelines the copy with the first DMA
                    # and roughly halves the copy->DMA tail after the very
                    # last matmul.
                    oc = MT_ // 2
                    last = (g == NG_ - 1 and d_i == DT - 1 and m == nm - 1)
                    # On the final chunk, fire the two half-DMAs from
                    # different queues so their ~600ns trigger executions
                    # overlap instead of serializing on the tail.
                    oq = [nc.sync, nc.scalar] if last else [nc.sync, nc.sync]
                    for k in range(2):
                        if last and k == 1:
                            # Copy the second half on the scalar engine so
                            # both copies (and both DMA triggers) overlap on
                            # the critical tail after the final matmul.
                            nc.scalar.copy(
                                o_sb[:, k * oc:(k + 1) * oc],
                                o_ps[:, k * oc:(k + 1) * oc])
                        else:
                            nc.vector.tensor_copy(
                                o_sb[:, k * oc:(k + 1) * oc],
                                o_ps[:, k * oc:(k + 1) * oc])
                        oq[k].dma_start(
                            out=ot[d_i][:, s0 + m * MT_ + k * oc:
                                        s0 + m * MT_ + (k + 1) * oc],
                            in_=o_sb[:, k * oc:(k + 1) * oc])

    nc.compile()
    if check_waits:
        skip = ("InstDrain", "InstEventSemaphore")
        bad = []
        for f in nc.m.functions:
            for bb in f.blocks:
                for inst in bb.instructions:
                    if type(inst).__name__ in skip or inst.sync_info is None:
                        continue
                    nw = len(inst.sync_info.on_wait or [])
                    if nw > 1:
                        bad.append((inst.name, type(inst).__name__, nw))
        if bad:
            raise RuntimeError(f"insts with >1 wait: {bad[:8]}")
    return nc


_NC_CACHE = {}


def _get_nc():
    if "nc" not in _NC_CACHE:
        _NC_CACHE["nc"] = build_nc()
    return _NC_CACHE["nc"]


def make_in_maps(hidden_states, w1, w2, w3, langs):
    hs = np.asarray(hidden_states, np.float32)
    w1 = np.asarray(w1, np.float32)
    w2 = np.asarray(w2, np.float32)
    w3 = np.asarray(w3, np.float32)
    langs = np.asarray(langs)
    DT, FT = D // P, F // P
    in_maps = []
    for b in range(B):
        e = int(langs[b, 0] - LANG_BASE) % E
        xtb = np.ascontiguousarray(hs[b].T.astype(bf16)).reshape(DT, P, S)
        w1b = np.ascontiguousarray(
            w1[e].reshape(DT, P, FT, P).transpose(2, 1, 0, 3).astype(bf16))
        w3b = np.ascontiguousarray(
            w3[e].reshape(DT, P, FT, P).transpose(2, 1, 0, 3).astype(bf16))
        w2b = np.ascontiguousarray(
            w2[e].reshape(FT, P, DT, P).transpose(2, 1, 0, 3).astype(bf16))
        in_maps.append({"xt": xtb, "w1": w1b, "w3": w3b, "w2": w2b})
    return in_maps


def assemble_output(results):
    out = np.empty((B, S, D), np.float32)
    for b in range(B):
        out[b] = results[b]["ot"].reshape(D, S).T
    return out


def kernel(hidden_states, w1, w2, w3, langs, **kw):
    nc = _get_nc()
    in_maps = make_in_maps(hidden_states, w1, w2, w3, langs)
    res = run_bass_kernel_spmd(nc, in_maps, list(range(8)))
    return assemble_output(res.results)


if __name__ == "__main__":
    rng = np.random.default_rng(0)
    hs = rng.standard_normal((B, S, D)).astype(np.float32)
    w1_ = (rng.standard_normal((E, D, F)) / np.sqrt(D)).astype(np.float32)
    w3_ = (rng.standard_normal((E, D, F)) / np.sqrt(D)).astype(np.float32)
    w2_ = (rng.standard_normal((E, F, D)) / np.sqrt(F)).astype(np.float32)
    langs = rng.integers(4, 12, (B, 1)).astype(np.int64)
    out = kernel(hs, w1_, w2_, w3_, langs)
    print(out.shape, out.dtype)


# revision 41
# speedup vs baseline: 1.0038x; 1.0001x over previous
"""MBart expert-layer (MoE routing) kernel for 8 Trainium2 NeuronCores.

Strategy: data-parallel over batch. Each batch row routes to exactly one
expert (lang code), so the expert gather happens on host (langs are host
data).  Core b computes a dense SwiGLU MLP for row b:
    out = (gelu(x @ W1) * (x @ W3)) @ W2
All device work happens in transposed orientation (activations stored
[d_model, seq]) so both matmul stages use the natural [K, M] weight layouts
as the stationary operand and no on-device transposes are needed.
Matmul inputs are bf16 (fp32 accumulate in PSUM); gelu/mul in fp32.

The TPB ISA allows one sync wait per instruction and this walrus build
refuses multi-wait instructions, so the module is built as bacc.Bacc and
nc.compile() runs bacc's generate_event_semaphores pass, which splits
excess waits into event-semaphore chains.  build_nc() asserts the
resulting ≤1-wait invariant.
"""

import numpy as np
import ml_dtypes
from contextlib import ExitStack

import concourse.bass as bass
import concourse.bacc as bacc
import concourse.mybir as mybir
from concourse.tile import TileContext
from concourse.bass_utils import run_bass_kernel_spmd

E, B, S, D, F = 8, 8, 2048, 1024, 4096
LANG_BASE = 4
P = 128
MT = 512          # matmul moving free dim (seq chunk)
NG = 2            # seq super-chunks; weights streamed NG times
BF16 = mybir.dt.bfloat16
F32 = mybir.dt.float32
bf16 = ml_dtypes.bfloat16


def build_nc(S_=S, D_=D, F_=F, MT_=MT, NG_=NG,
             act=mybir.ActivationFunctionType.Gelu, check_waits=True):
    DT, FT = D_ // P, F_ // P
    sg = S_ // NG_
    nm = sg // MT_
    W2SUB = 8 if FT % 8 == 0 else 1   # w2 block split into sub-DMAs per d_i
    FS = FT // W2SUB                  # f-tiles per w2 sub-block
    nc = bacc.Bacc()
    xt = nc.declare_dram_parameter("xt", [DT, P, S_], BF16, isOutput=False)
    w1 = nc.declare_dram_parameter("w1", [FT, P, DT, P], BF16, isOutput=False)
    w3 = nc.declare_dram_parameter("w3", [FT, P, DT, P], BF16, isOutput=False)
    w2 = nc.declare_dram_parameter("w2", [DT, P, FT, P], BF16, isOutput=False)
    ot = nc.declare_dram_parameter("ot", [DT, P, S_], F32, isOutput=True)

    with TileContext(nc) as tc, ExitStack() as ctx:
        xpool = ctx.enter_context(tc.tile_pool(name="x", bufs=1))
        wpool = ctx.enter_context(tc.tile_pool(name="w", bufs=4))
        w2pool = ctx.enter_context(tc.tile_pool(name="w2", bufs=2))
        hpool = ctx.enter_context(tc.tile_pool(name="h", bufs=1))
        gpool = ctx.enter_context(tc.tile_pool(name="g", bufs=3))
        opool = ctx.enter_context(tc.tile_pool(name="o", bufs=3))
        ppool = ctx.enter_context(tc.tile_pool(name="ps", bufs=2, space="PSUM"))
        p2pool = ctx.enter_context(tc.tile_pool(name="ps2", bufs=2, space="PSUM"))

        x_sb = []
        for d_i in range(DT):
            t = xpool.tile([P, S_], BF16, name=f"x{d_i}", tag=f"x{d_i}")
            x_sb.append(t)
        # Load the first super-chunk's columns of every x tile before the
        # rest so the first matmuls stop waiting on the full 4MB transfer.
        # Each dma_start costs ~600ns of issuing-queue time (DIRECT2D), so
        # eight triggers on one queue serialize over ~5us -- spread the
        # first super-chunk's triggers across the queues that are idle at
        # kernel start (vector/scalar) to get all eight transfers in
        # flight by ~7.5us instead of ~10.5us.
        first_q = [nc.sync, nc.sync, nc.sync, nc.sync, nc.scalar,
                   nc.scalar, nc.scalar, nc.scalar]
        for d_i in range(DT):
            first_q[d_i % len(first_q)].dma_start(
                out=x_sb[d_i][:, 0:sg], in_=xt[d_i][:, 0:sg])


        for g in range(NG_):
            s0 = g * sg
            # ---- phase A: hT[f, m] = gelu(W1.T x) * (W3.T x) ----
            h_tiles = []
            for f_i in range(FT):
                w1_t = wpool.tile([P, DT, P], BF16, name="w1t", tag="w1t")
                w3_t = wpool.tile([P, DT, P], BF16, name="w3t", tag="w3t")
                # Halved weight transfers lower the instantaneous SBUF
                # write burst while a tile lands (suspected source of
                # periodic PE operand-fetch stalls).
                hd = DT // 2
                nc.gpsimd.dma_start(out=w1_t[:, :hd, :], in_=w1[f_i][:, :hd, :])
                nc.gpsimd.dma_start(out=w1_t[:, hd:, :], in_=w1[f_i][:, hd:, :])
                nc.gpsimd.dma_start(out=w3_t[:, :hd, :], in_=w3[f_i][:, :hd, :])
                nc.gpsimd.dma_start(out=w3_t[:, hd:, :], in_=w3[f_i][:, hd:, :])
                h_sb = hpool.tile([P, sg], BF16, name=f"h{f_i}", tag=f"h{f_i}")
                for m in range(nm):
                    ms = s0 + m * MT_
                    a_ps = ppool.tile([P, MT_], F32, name="a_ps", tag="a")
                    b_ps = ppool.tile([P, MT_], F32, name="b_ps", tag="b")
                    # b (w3 path) first so the gelu is the latest producer
                    # feeding the h-mul: the wait legalizer can then anchor
                    # the mul's PE wait on the gelu at zero cost.
                    # For the first f-iteration, accumulate in x ARRIVAL
                    # order (d0-d3 stream via sync, d4-d7 via scalar, so
                    # chunks land interleaved): consuming in-order leaves
                    # micro-gaps that keep the early-stream busy fraction
                    # under the HAM un-throttle threshold on unlucky runs.
                    if g == 0 and f_i == 0:
                        dord = [0, 4, 1, 5, 2, 6, 3, 7][:DT] if DT == 8                             else list(range(DT))
                    else:
                        dord = list(range(DT))
                    for i, d_i in enumerate(dord):
                        nc.tensor.matmul(
                            b_ps[:], w3_t[:, d_i, :], x_sb[d_i][:, ms:ms + MT_],
                            start=(i == 0), stop=(i == DT - 1))
                    for i, d_i in enumerate(dord):
                        nc.tensor.matmul(
                            a_ps[:], w1_t[:, d_i, :], x_sb[d_i][:, ms:ms + MT_],
                            start=(i == 0), stop=(i == DT - 1))
                    g_sb = gpool.tile([P, MT_], F32, name="g_sb", tag="g")
                    nc.scalar.activation(g_sb[:], a_ps[:], act)
                    # Later seq super-chunks of x are triggered one-per-gelu
                    # here: the scalar queue reaches these only after the
                    # first gelu (~17.5us), so the g1 transfers stop
                    # fair-sharing the DMA rings with the critical first
                    # 2.5MB (they were slowing it ~45%; the first matmul is
                    # purely data-gated on that set).
                    if g == 0 and 2 * f_i + m < 8 * (NG_ - 1):
                        slot = 2 * f_i + m
                        g2, d2 = 1 + slot // DT, slot % DT
                        nc.scalar.dma_start(
                            out=x_sb[d2][:, g2 * sg:(g2 + 1) * sg],
                            in_=xt[d2][:, g2 * sg:(g2 + 1) * sg])
                    nc.vector.tensor_mul(
                        h_sb[:, m * MT_:(m + 1) * MT_], g_sb[:], b_ps[:])
                h_tiles.append(h_sb)
            # ---- phase B: outT[d, m] = W2.T hT ----
            for d_i in range(DT):
                w2_ts = []
                for k in range(W2SUB):
                    w2_t = w2pool.tile([P, FS, P], BF16, name=f"w2t{k}",
                                       tag=f"w2t{k}")
                    nc.gpsimd.dma_start(
                        out=w2_t[:], in_=w2[d_i][:, k * FS:(k + 1) * FS, :])
                    w2_ts.append(w2_t)
                for m in range(nm):
                    o_ps = p2pool.tile([P, MT_], F32, name="o_ps", tag="o")
                    for f_i in range(FT):
                        nc.tensor.matmul(
                            o_ps[:], w2_ts[f_i // FS][:, f_i % FS, :],
                            h_tiles[f_i][:, m * MT_:(m + 1) * MT_],
                            start=(f_i == 0), stop=(f_i == FT - 1))
                    o_sb = opool.tile([P, MT_], F32, name="o_sb", tag="osb")
                    # Two-way split pipelines the copy with the first DMA
                    # and roughly halves the copy->DMA tail after the very
                    # last matmul.
                    oc = MT_ // 2
                    last = (g == NG_ - 1 and d_i == DT - 1 and m == nm - 1)
                    # On the final chunk, fire the two half-DMAs from
                    # different queues so their ~600ns trigger executions
                    # overlap instead of serializing on the tail.
                    oq = [nc.sync, nc.scalar] if last else [nc.sync, nc.sync]
                    for k in range(2):
                        if last and k == 1:
                            # Copy the second half on the scalar engine so
                            # both copies (and both DMA triggers) overlap on
                            # the critical tail after the final matmul.
                            nc.scalar.copy(
                                o_sb[:, k * oc:(k + 1) * oc],
                                o_ps[:, k * oc:(k + 1) * oc])
                        else:
                            nc.vector.tensor_copy(
                                o_sb[:, k * oc:(k + 1) * oc],
                                o_ps[:, k * oc:(k + 1) * oc])
                        oq[k].dma_start(
                            out=ot[d_i][:, s0 + m * MT_ + k * oc:
                                        s0 + m * MT_ + (k + 1) * oc],
                            in_=o_sb[:, k * oc:(k + 1) * oc])

    nc.compile()
    if check_waits:
        skip = ("InstDrain", "InstEventSemaphore")
        bad = []
        for f in nc.m.functions:
            for bb in f.blocks:
                for inst in bb.instructions:
                    if type(inst).__name__ in skip or inst.sync_info is None:
                        continue
                    nw = len(inst.sync_info.on_wait or [])
                    if nw > 1:
                        bad.append((inst.name, type(inst).__name__, nw))
        if bad:
            raise RuntimeError(f"insts with >1 wait: {bad[:8]}")
    return nc


_NC_CACHE = {}


def _get_nc():
    if "nc" not in _NC_CACHE:
        _NC_CACHE["nc"] = build_nc()
    return _NC_CACHE["nc"]


def make_in_maps(hidden_states, w1, w2, w3, langs):
    hs = np.asarray(hidden_states, np.float32)
    w1 = np.asarray(w1, np.float32)
    w2 = np.asarray(w2, np.float32)
    w3 = np.asarray(w3, np.float32)
    langs = np.asarray(langs)
    DT, FT = D // P, F // P
    in_maps = []
    for b in range(B):
        e = int(langs[b, 0] - LANG_BASE) % E
        xtb = np.ascontiguousarray(hs[b].T.astype(bf16)).reshape(DT, P, S)
        w1b = np.ascontiguousarray(
            w1[e].reshape(DT, P, FT, P).transpose(2, 1, 0, 3).astype(bf16))
        w3b = np.ascontiguousarray(
            w3[e].reshape(DT, P, FT, P).transpose(2, 1, 0, 3).astype(bf16))
        w2b = np.ascontiguousarray(
            w2[e].reshape(FT, P, DT, P).transpose(2, 1, 0, 3).astype(bf16))
        in_maps.append({"xt": xtb, "w1": w1b, "w3": w3b, "w2": w2b})
    return in_maps


def assemble_output(results):
    out = np.empty((B, S, D), np.float32)
    for b in range(B):
        out[b] = results[b]["ot"].reshape(D, S).T
    return out


def kernel(hidden_states, w1, w2, w3, langs, **kw):
    nc = _get_nc()
    in_maps = make_in_maps(hidden_states, w1, w2, w3, langs)
    res = run_bass_kernel_spmd(nc, in_maps, list(range(8)))
    return assemble_output(res.results)


if __name__ == "__main__":
    rng = np.random.default_rng(0)
    hs = rng.standard_normal((B, S, D)).astype(np.float32)
    w1_ = (rng.standard_normal((E, D, F)) / np.sqrt(D)).astype(np.float32)
    w3_ = (rng.standard_normal((E, D, F)) / np.sqrt(D)).astype(np.float32)
    w2_ = (rng.standard_normal((E, F, D)) / np.sqrt(F)).astype(np.float32)
    langs = rng.integers(4, 12, (B, 1)).astype(np.int64)
    out = kernel(hs, w1_, w2_, w3_, langs)
    print(out.shape, out.dtype)
